# revision 26
# baseline (speedup 1.0000x reference)
"""LocalSpatialEncoding (RandLA-Net) Bass/Tile kernel for Trainium2, 8-core SPMD.

Math (per batch b, full N points, K neighbors, D=64 output channels):
  u_j = [center(3), nbr(3), center-nbr(3), dist(1)]  for j=(n,k)
  x   = relu(GN16(conv1x1(u) + conv_b))              -> channels 0..63
  out = concat([x, gathered features], channel dim)  -> (B, 128, N, K)

Folding: with conv_w = [Wc | Wg | Wd | w9] (10 cols),
  x_raw = A@c + Bm@g + w9*dist,  A = Wc+Wd, Bm = Wg-Wd  (bias folded into GN)

Sharding: N split across 8 cores (Ns = N/8 per core, both batches on every
core).  Neighbor indices are global, so the neighbor-feature half of the
output is produced by a hardware-DGE transposed dma_gather straight from a
point-major DRAM plane fp[N, 128] (f16 row n = both batches' 64 features of
point n): each index fetches one 256B row and the crossbar delivers it
channel-major into SBUF, so one gather call per tile yields output-ready
[64, T] slabs for both batches at DMA-engine speed (the GPSIMD ap_gather
tops out ~25x slower and dominated earlier revisions).

The 7-row matmul rhs (center/neighbor coords + dist) is precomputed on the
host as an exact f16 hi/lo split, duplicated into the 21-row layout
[v_hi; v_hi; v_lo] so a single f16 matmul against [W_hi; W_lo; W_hi]
accumulates the exact-fp32 conv output in PSUM.  Pass A streams it to get
per-channel sum/sumsq for GroupNorm (S via DVE reduce of PSUM, Q via ACT
Square accumulate), a 2KB AllReduce combines stats across cores (its
latency hides behind the gather stream), and pass B re-matmuls the same
rhs and applies the folded affine+ReLU in one activation pass.  All HBM
traffic is f16; final f16 rounding is ~5e-4 relative vs the 2e-2 gate.
"""

import sys
from contextlib import ExitStack

import numpy as np

sys.path.insert(0, "/opt/trn_rl_repo")

import concourse.bass as bass  # noqa: E402
import concourse.bacc as bacc  # noqa: E402
import concourse.mybir as mybir  # noqa: E402
import concourse.tile as tile  # noqa: E402

F32 = mybir.dt.float32
F16 = mybir.dt.float16
I16 = mybir.dt.int16

B = 2
D = 64
GROUPS = 16
EPS = 1e-6


def build_program(N, NS, K, TILE, n_cores):
    """Build the SPMD Bass program (identical on all cores).

    Per-core inputs:
      fp    [N, 2D]      f16: point-major feature plane, row n =
                              [feat_b0[:,n], feat_b1[:,n]] (replicated)
      idxg  [B, 2D, J/16] i16: wrapped neighbor indices (idx[j] at
                              [j%16, j//16]), replicated across the 8
                              16-partition groups
      rhs21 [B, 21, J]   f16: matmul rhs rows [v_hi(7); v_hi(7); v_lo(7)],
                              v = [center(3); nbr(3); dist(1)]
      wd21  [21, D]      f16: lhsT = [W_hi; W_lo; W_hi]
      misc  [D, 4]       f32: cols = conv_b, gamma, beta, pad
      g1    [D, GROUPS]  f32: channel->group indicator
      g2    [GROUPS, D]  f32: group->channel indicator
    Output:
      out   [B, 2D, NS, K] f16 (this core's N-shard of the full output)
    """
    J = NS * K          # columns per batch per core
    NT = J // TILE      # tiles per batch
    PTS = TILE // K     # points per tile
    CHUNK = min(2048, TILE)  # ACT/DVE granularity (PSUM tile width)
    NC = TILE // CHUNK
    GATHER_CHUNK = 512  # num_idxs >= 2048 per transposed dma_gather wedges hw
    MTOT = float(N * K)

    nc = bacc.Bacc(
        "TRN2", target_bir_lowering=False, debug=False, num_devices=n_cores
    )

    fp = nc.dram_tensor("fp", [N, 2 * D], F16, kind="ExternalInput").ap()
    idxg = nc.dram_tensor("idxg", [B, 2 * D, J // 16], I16, kind="ExternalInput").ap()
    rhs = nc.dram_tensor("rhs21", [B, 21, J], F16, kind="ExternalInput").ap()
    wd21 = nc.dram_tensor("wd21", [21, D], F16, kind="ExternalInput").ap()
    misc = nc.dram_tensor("misc", [D, 4], F32, kind="ExternalInput").ap()
    g1d = nc.dram_tensor("g1", [D, GROUPS], F32, kind="ExternalInput").ap()
    g2d = nc.dram_tensor("g2", [GROUPS, D], F32, kind="ExternalInput").ap()
    out = nc.dram_tensor("out", [B, 2 * D, NS, K], F16, kind="ExternalOutput").ap()

    with tile.TileContext(nc) as tc, ExitStack() as ctx:
        const_pool = ctx.enter_context(tc.tile_pool(name="const", bufs=1))
        idx_pool = ctx.enter_context(tc.tile_pool(name="idxp", bufs=1))
        gath_pool = ctx.enter_context(tc.tile_pool(name="gathp", bufs=4))
        vt_pool = ctx.enter_context(tc.tile_pool(name="vtp", bufs=3))
        xo_pool = ctx.enter_context(tc.tile_pool(name="xop", bufs=2))
        dump_pool = ctx.enter_context(tc.tile_pool(name="dumpp", bufs=1))
        sq_pool = ctx.enter_context(tc.tile_pool(name="sqp", bufs=2))
        stat_pool = ctx.enter_context(tc.tile_pool(name="statp", bufs=1))
        psum_pool = ctx.enter_context(tc.tile_pool(name="psump", bufs=2, space="PSUM"))
        dram_pool = ctx.enter_context(tc.tile_pool(name="dramp", bufs=1, space="DRAM"))

        # --- constants ---
        wd_sb = const_pool.tile([21, D], F16)
        nc.sync.dma_start(wd_sb[:], wd21[:])
        misc_sb = const_pool.tile([D, 4], F32)
        nc.sync.dma_start(misc_sb[:], misc[:])
        g1_sb = const_pool.tile([D, GROUPS], F32)
        nc.sync.dma_start(g1_sb[:], g1d[:])
        g2_sb = const_pool.tile([GROUPS, D], F32)
        nc.sync.dma_start(g2_sb[:], g2d[:])
        idx_sb = idx_pool.tile([2 * D, B * (J // 16)], I16)
        for b in range(B):
            nc.sync.dma_start(
                idx_sb[:, b * (J // 16) : (b + 1) * (J // 16)], idxg[b]
            )

        b_col = misc_sb[:, 0:1]
        gam_col = misc_sb[:, 1:2]
        bet_col = misc_sb[:, 2:3]

        # per-(b,tile,chunk) stats columns: S = sum x, Q = sum x^2 per channel
        NCOL = B * NT * NC
        statsS = stat_pool.tile([D, NCOL], F32)
        statsQ = stat_pool.tile([D, NCOL], F32)
        dump = dump_pool.tile([D, CHUNK], F32)

        # ---------------- pass A: stats (no gathers in this stream) ------
        for b in range(B):
            for t in range(NT):
                jslc = slice(t * TILE, (t + 1) * TILE)
                vt = vt_pool.tile([21, TILE], F16, tag="vt")
                nc.sync.dma_start(vt[:, :], rhs[b, :, jslc])
                for c in range(NC):
                    ps = psum_pool.tile([D, CHUNK], F32, tag="ps")
                    for q in range(CHUNK // 512):
                        cq = slice(c * CHUNK + q * 512, c * CHUNK + (q + 1) * 512)
                        nc.tensor.matmul(
                            ps[:, q * 512 : (q + 1) * 512],
                            lhsT=wd_sb[:, :],
                            rhs=vt[:, cq],
                            start=True,
                            stop=True,
                        )
                    col = (b * NT + t) * NC + c
                    nc.vector.tensor_reduce(
                        statsS[:, col : col + 1],
                        ps[:, :],
                        axis=mybir.AxisListType.X,
                        op=mybir.AluOpType.add,
                    )
                    nc.scalar.activation(
                        dump[:, :],
                        ps[:, :],
                        mybir.ActivationFunctionType.Square,
                        accum_out=statsQ[:, col : col + 1],
                    )

        # ---------------- stats finalize ----------------
        sqy = stat_pool.tile([D, 4], F32)  # cols: S_b0, S_b1, Q_b0, Q_b1 (local)
        for b in range(B):
            csl = slice(b * NT * NC, (b + 1) * NT * NC)
            nc.vector.tensor_reduce(
                sqy[:, b : b + 1], statsS[:, csl],
                axis=mybir.AxisListType.X, op=mybir.AluOpType.add,
            )
            nc.vector.tensor_reduce(
                sqy[:, 2 + b : 3 + b], statsQ[:, csl],
                axis=mybir.AxisListType.X, op=mybir.AluOpType.add,
            )
        arin = dram_pool.tile([D, 4], F32)
        arout = dram_pool.tile([D, 4], F32)
        nc.sync.dma_start(arin[:], sqy[:, :])

        # ------- feature-gather stream (Pool + Sync queues, independent) --
        # The AllReduce trigger is interleaved into the gather stream so the
        # Pool queue reaches it right around the time the stats finish; pass B
        # (on the Act HWDGE queue) then overlaps the remaining gathers.
        # Per-call num_idxs is ring-limited (>= 1024 wedges the hw; 768 and
        # 512 are probe-verified).  Use 768-idx calls to amortize the ~1us
        # fixed SWDGE cost, with one 256-idx remainder call per batch.
        GCMAX = 768 if TILE >= 2048 else min(GATHER_CHUNK, TILE)
        chunks = []  # (batch, start_idx_in_J, num_idxs)
        for b in range(B):
            j0 = 0
            while j0 < J:
                gc_n = min(GCMAX, J - j0)
                chunks.append((b, j0, gc_n))
                j0 += gc_n
        CC_AT = 38  # gather calls issued before the AllReduce trigger

        def emit_gather(b, j0, gc_n):
            gth = gath_pool.tile([2 * D, gc_n], F16, tag="gth")
            nc.gpsimd.dma_gather(
                gth[:, :].rearrange("p (o n) -> p o n", o=1),
                fp[:, :],
                idx_sb[:, b * (J // 16) + j0 // 16
                       : b * (J // 16) + (j0 + gc_n) // 16],
                num_idxs=gc_n,
                num_idxs_reg=gc_n,
                elem_size=2 * D,
                transpose=True,
            )
            nc.sync.dma_start(
                out[b, D : 2 * D, j0 // K : (j0 + gc_n) // K, :],
                gth[b * D : (b + 1) * D, :],
            )

        for b, j0, gc_n in chunks[:CC_AT]:
            emit_gather(b, j0, gc_n)
        nc.gpsimd.collective_compute(
            "AllReduce",
            mybir.AluOpType.add,
            replica_groups=[list(range(n_cores))],
            ins=[arin.opt()],
            outs=[arout.opt()],
        )
        for b, j0, gc_n in chunks[CC_AT:]:
            emit_gather(b, j0, gc_n)

        sq_g = stat_pool.tile([D, 4], F32)  # global S_b0, S_b1, Q_b0, Q_b1
        nc.scalar.dma_start(sq_g[:], arout[:])

        # with bias folded:  Sy = S + M*b ; Qy = Q + b*(M*b + 2S)
        sqy2 = stat_pool.tile([D, 4], F32)  # Sy_b0, Sy_b1, Qy_b0, Qy_b1
        s2 = stat_pool.tile([D, 2], F32)
        tmp1 = stat_pool.tile([D, 2], F32)
        for b in range(B):
            S_b = sq_g[:, b : b + 1]
            Q_b = sq_g[:, 2 + b : 3 + b]
            nc.scalar.activation(
                sqy2[:, b : b + 1], b_col,
                mybir.ActivationFunctionType.Identity, bias=S_b, scale=MTOT,
            )
            nc.vector.tensor_add(s2[:, b : b + 1], S_b, S_b)
            nc.scalar.activation(
                tmp1[:, b : b + 1], b_col,
                mybir.ActivationFunctionType.Identity,
                bias=s2[:, b : b + 1], scale=MTOT,
            )
            nc.vector.tensor_mul(tmp1[:, b : b + 1], tmp1[:, b : b + 1], b_col)
            nc.vector.tensor_add(sqy2[:, 2 + b : 3 + b], Q_b, tmp1[:, b : b + 1])

        # group sums: gs[16, 4] = g1^T @ sqy2
        gps = psum_pool.tile([GROUPS, 4], F32, tag="ps")
        nc.tensor.matmul(gps[:, :], lhsT=g1_sb[:, :], rhs=sqy2[:, :], start=True, stop=True)
        mue = stat_pool.tile([GROUPS, 4], F32)  # cols 0-1: mu; 2-3: E2 then rs
        inv4m = 1.0 / (4.0 * MTOT)
        nc.scalar.activation(mue[:, :], gps[:, :], mybir.ActivationFunctionType.Copy, scale=inv4m)
        musq = stat_pool.tile([GROUPS, 2], F32)
        nc.scalar.activation(musq[:, :], mue[:, 0:2], mybir.ActivationFunctionType.Square)
        var = stat_pool.tile([GROUPS, 2], F32)
        nc.vector.tensor_sub(var[:, :], mue[:, 2:4], musq[:, :])
        nc.vector.tensor_scalar_add(var[:, :], var[:, :], EPS)
        nc.vector.reciprocal(var[:, :], var[:, :])
        nc.scalar.activation(mue[:, 2:4], var[:, :], mybir.ActivationFunctionType.Sqrt)

        # broadcast groups -> channels: mr64[64, 4] = g2^T @ mue
        mps = psum_pool.tile([D, 4], F32, tag="ps")
        nc.tensor.matmul(mps[:, :], lhsT=g2_sb[:, :], rhs=mue[:, :], start=True, stop=True)
        mr64 = stat_pool.tile([D, 4], F32)
        nc.scalar.activation(mr64[:, :], mps[:, :], mybir.ActivationFunctionType.Copy)

        # final per-channel scale s = gamma*rs, shift t = (b - mu)*s + beta
        sc = stat_pool.tile([D, 2], F32)
        tc_ = stat_pool.tile([D, 2], F32)
        for b in range(B):
            nc.vector.tensor_mul(sc[:, b : b + 1], mr64[:, 2 + b : 3 + b], gam_col)
            nc.vector.tensor_sub(tc_[:, b : b + 1], b_col, mr64[:, b : b + 1])
            nc.vector.tensor_mul(tc_[:, b : b + 1], tc_[:, b : b + 1], sc[:, b : b + 1])
            nc.vector.tensor_add(tc_[:, b : b + 1], tc_[:, b : b + 1], bet_col)

        # ---------------- pass B: x = relu(s*x_raw + t) ----------------
        # All pass-B DMAs ride the Act HWDGE queue: the Sync queue is busy
        # draining gather->out writes and would head-of-line block pass B.
        for b in range(B):
            for t in range(NT):
                jslc = slice(t * TILE, (t + 1) * TILE)
                vt = vt_pool.tile([21, TILE], F16, tag="vt")
                nc.scalar.dma_start(vt[:, :], rhs[b, :, jslc])
                xo = xo_pool.tile([D, TILE], F16, tag="xo")
                for c in range(NC):
                    ps = psum_pool.tile([D, CHUNK], F32, tag="ps")
                    for q in range(CHUNK // 512):
                        cq = slice(c * CHUNK + q * 512, c * CHUNK + (q + 1) * 512)
                        nc.tensor.matmul(
                            ps[:, q * 512 : (q + 1) * 512],
                            lhsT=wd_sb[:, :],
                            rhs=vt[:, cq],
                            start=True,
                            stop=True,
                        )
                    nc.scalar.activation(
                        xo[:, c * CHUNK : (c + 1) * CHUNK],
                        ps[:, :],
                        mybir.ActivationFunctionType.Relu,
                        bias=tc_[:, b : b + 1],
                        scale=sc[:, b : b + 1],
                    )
                nc.scalar.dma_start(
                    out[b, 0:D, t * PTS : (t + 1) * PTS, :], xo[:, :]
                )

    nc.compile()
    return nc


def host_prep(coords, features, idx, dist, conv_w, conv_b, gn_gamma, gn_beta,
              N, NS, K, n_cores):
    """Full inputs -> list of per-core input maps."""
    coords = np.asarray(coords, dtype=np.float32)
    features = np.asarray(features, dtype=np.float32)
    idx = np.asarray(idx)
    dist = np.asarray(dist, dtype=np.float32)
    conv_w = np.asarray(conv_w, dtype=np.float32)
    conv_b = np.asarray(conv_b, dtype=np.float32)
    gn_gamma = np.asarray(gn_gamma, dtype=np.float32)
    gn_beta = np.asarray(gn_beta, dtype=np.float32)

    J = NS * K
    # point-major f16 feature plane: row n = [b0 features, b1 features]
    fp = np.ascontiguousarray(
        features[:, :, :, 0].astype(np.float16).transpose(2, 0, 1).reshape(N, 2 * D)
    )

    # weights: A = Wc + Wd, Bm = Wg - Wd, w9; lhsT rows = [A; Bm; w9]
    # matching the rhs row order [center(3); nbr(3); dist(1)]
    A = conv_w[:, 0:3] + conv_w[:, 6:9]
    Bm = conv_w[:, 3:6] - conv_w[:, 6:9]
    w9 = conv_w[:, 9:10]
    wb = np.concatenate([A.T, Bm.T, w9.T], axis=0).astype(np.float32)  # [7, 64]
    wh = wb.astype(np.float16)
    wl = (wb - wh.astype(np.float32)).astype(np.float16)
    wd21 = np.concatenate([wh, wl, wh], axis=0)  # [21, 64] f16

    misc = np.stack(
        [conv_b, gn_gamma, gn_beta, np.zeros_like(conv_b)], axis=1
    ).astype(np.float32)  # [64, 4]
    dgrp = np.arange(D) // (D // GROUPS)
    g1 = (dgrp[:, None] == np.arange(GROUPS)[None, :]).astype(np.float32)
    g2 = np.ascontiguousarray(g1.T)

    in_maps = []
    for c in range(n_cores):
        nsl = slice(c * NS, (c + 1) * NS)
        idx_c = idx[:, nsl, :]  # [B, NS, K]
        # wrapped int16 idx layout: index j at [j%16, j//16], replicated 8x
        idxf = idx_c.reshape(B, J)
        idxw16 = idxf.reshape(B, J // 16, 16).transpose(0, 2, 1).astype(np.int16)
        idxg = np.ascontiguousarray(np.tile(idxw16, (1, (2 * D) // 16, 1)))

        # exact f16 hi/lo split of the 7-row matmul rhs [c(3); g(3); d(1)],
        # duplicated into the 21-row [v_hi; v_hi; v_lo] matmul layout
        rhs21 = np.empty((B, 21, J), np.float16)
        for b in range(B):
            ctr = np.repeat(coords[b, nsl, :], K, axis=0).T      # [3, J]
            g = coords[b, idx_c[b].reshape(-1), :].T             # [3, J]
            d = dist[b, nsl, :].reshape(1, J)                    # [1, J]
            v7 = np.concatenate([ctr, g, d], axis=0)             # [7, J] f32
            vh = v7.astype(np.float16)
            vl = (v7 - vh.astype(np.float32)).astype(np.float16)
            rhs21[b, 0:7] = vh
            rhs21[b, 7:14] = vh
            rhs21[b, 14:21] = vl

        in_maps.append(
            {
                "fp": fp,
                "idxg": idxg,
                "rhs21": rhs21,
                "wd21": wd21,
                "misc": misc,
                "g1": g1,
                "g2": g2,
            }
        )
    return in_maps


def assemble(results, N, NS, K, n_cores):
    """Per-core 'out' shards (f16) -> full (B, 2D, N, K) f32."""
    return np.concatenate(
        [results[c]["out"] for c in range(n_cores)], axis=2
    ).astype(np.float32)


# ---------------------------------------------------------------------------
# self-contained entry point: full inputs -> full output on 8 NeuronCores
# ---------------------------------------------------------------------------
_N, _NS, _K, _TILE, _NCORES = 32768, 4096, 16, 2048, 8
_PROGRAM = None


def _get_program():
    global _PROGRAM
    if _PROGRAM is None:
        _PROGRAM = build_program(_N, _NS, _K, _TILE, _NCORES)
    return _PROGRAM


def kernel(coords, features, idx, dist, conv_w, conv_b, gn_gamma, gn_beta):
    nc = _get_program()
    in_maps = host_prep(
        coords, features, idx, dist, conv_w, conv_b, gn_gamma, gn_beta,
        _N, _NS, _K, _NCORES,
    )
    from concourse.bass_utils import run_bass_kernel_spmd

    res = run_bass_kernel_spmd(nc, in_maps, list(range(_NCORES)))
    return assemble(res.results, _N, _NS, _K, _NCORES)


# revision 27
# speedup vs baseline: 1.1638x; 1.1638x over previous
"""LocalSpatialEncoding (RandLA-Net) Bass/Tile kernel for Trainium2, 8-core SPMD.

Math (per batch b, full N points, K neighbors, D=64 output channels):
  u_j = [center(3), nbr(3), center-nbr(3), dist(1)]  for j=(n,k)
  x   = relu(GN16(conv1x1(u) + conv_b))              -> channels 0..63
  out = concat([x, gathered features], channel dim)  -> (B, 128, N, K)

Folding: with conv_w = [Wc | Wg | Wd | w9] (10 cols),
  x_raw = A@c + Bm@g + w9*dist,  A = Wc+Wd, Bm = Wg-Wd  (bias folded into GN)

Sharding: N split across 8 cores (Ns = N/8 per core, both batches on every
core).  Neighbor indices are global, so the neighbor-feature half of the
output is produced by a hardware-DGE transposed dma_gather straight from a
point-major DRAM plane fp[N, 128] (f16 row n = both batches' 64 features of
point n): each index fetches one 256B row and the crossbar delivers it
channel-major into SBUF, so one gather call per tile yields output-ready
[64, T] slabs for both batches at DMA-engine speed (the GPSIMD ap_gather
tops out ~25x slower and dominated earlier revisions).

The 7-row matmul rhs (center/neighbor coords + dist) is precomputed on the
host as an exact f16 hi/lo split, duplicated into the 21-row layout
[v_hi; v_hi; v_lo] so a single f16 matmul against [W_hi; W_lo; W_hi]
accumulates the exact-fp32 conv output in PSUM.  Pass A streams it to get
per-channel sum/sumsq for GroupNorm (S via DVE reduce of PSUM, Q via ACT
Square accumulate), a 2KB AllReduce combines stats across cores (its
latency hides behind the gather stream), and pass B re-matmuls the same
rhs and applies the folded affine+ReLU in one activation pass.  All HBM
traffic is f16; final f16 rounding is ~5e-4 relative vs the 2e-2 gate.
"""

import sys
from contextlib import ExitStack

import numpy as np

sys.path.insert(0, "/opt/trn_rl_repo")

import concourse.bass as bass  # noqa: E402
import concourse.bacc as bacc  # noqa: E402
import concourse.mybir as mybir  # noqa: E402
import concourse.tile as tile  # noqa: E402

F32 = mybir.dt.float32
F16 = mybir.dt.float16
I16 = mybir.dt.int16

B = 2
D = 64
GROUPS = 16
EPS = 1e-6


def build_program(N, NS, K, TILE, n_cores):
    """Build the SPMD Bass program (identical on all cores).

    Per-core inputs:
      fp    [N, 2D]      f16: point-major feature plane, row n =
                              [feat_b0[:,n], feat_b1[:,n]] (replicated)
      idxg  [B, 2D, J/16] i16: wrapped neighbor indices (idx[j] at
                              [j%16, j//16]), replicated across the 8
                              16-partition groups
      rhs21 [B, 21, J]   f16: matmul rhs rows [v_hi(7); v_hi(7); v_lo(7)],
                              v = [center(3); nbr(3); dist(1)]
      wd21  [21, D]      f16: lhsT = [W_hi; W_lo; W_hi]
      misc  [D, 4]       f32: cols = conv_b, gamma, beta, pad
      g1    [D, GROUPS]  f32: channel->group indicator
      g2    [GROUPS, D]  f32: group->channel indicator
    Output:
      out   [B, 2D, NS, K] f16 (this core's N-shard of the full output)
    """
    J = NS * K          # columns per batch per core
    NT = J // TILE      # tiles per batch
    PTS = TILE // K     # points per tile
    CHUNK = min(2048, TILE)  # ACT/DVE granularity (PSUM tile width)
    NC = TILE // CHUNK
    GATHER_CHUNK = 512  # num_idxs >= 2048 per transposed dma_gather wedges hw
    MTOT = float(N * K)

    nc = bacc.Bacc(
        "TRN2", target_bir_lowering=False, debug=False, num_devices=n_cores
    )

    fp = nc.dram_tensor("fp", [N, 2 * D], F16, kind="ExternalInput").ap()
    idxg = nc.dram_tensor("idxg", [B, 2 * D, J // 16], I16, kind="ExternalInput").ap()
    rhs = nc.dram_tensor("rhs21", [B, 21, J], F16, kind="ExternalInput").ap()
    wd21 = nc.dram_tensor("wd21", [21, D], F16, kind="ExternalInput").ap()
    misc = nc.dram_tensor("misc", [D, 4], F32, kind="ExternalInput").ap()
    g1d = nc.dram_tensor("g1", [D, GROUPS], F32, kind="ExternalInput").ap()
    g2d = nc.dram_tensor("g2", [GROUPS, D], F32, kind="ExternalInput").ap()
    out = nc.dram_tensor("out", [B, 2 * D, NS, K], F16, kind="ExternalOutput").ap()

    with tile.TileContext(nc) as tc, ExitStack() as ctx:
        const_pool = ctx.enter_context(tc.tile_pool(name="const", bufs=1))
        idx_pool = ctx.enter_context(tc.tile_pool(name="idxp", bufs=1))
        gath_pool = ctx.enter_context(tc.tile_pool(name="gathp", bufs=4))
        vt_pool = ctx.enter_context(tc.tile_pool(name="vtp", bufs=3))
        xo_pool = ctx.enter_context(tc.tile_pool(name="xop", bufs=2))
        dump_pool = ctx.enter_context(tc.tile_pool(name="dumpp", bufs=1))
        sq_pool = ctx.enter_context(tc.tile_pool(name="sqp", bufs=2))
        stat_pool = ctx.enter_context(tc.tile_pool(name="statp", bufs=1))
        psum_pool = ctx.enter_context(tc.tile_pool(name="psump", bufs=2, space="PSUM"))
        dram_pool = ctx.enter_context(tc.tile_pool(name="dramp", bufs=1, space="DRAM"))

        # --- constants ---
        wd_sb = const_pool.tile([21, D], F16)
        nc.sync.dma_start(wd_sb[:], wd21[:])
        misc_sb = const_pool.tile([D, 4], F32)
        nc.sync.dma_start(misc_sb[:], misc[:])
        g1_sb = const_pool.tile([D, GROUPS], F32)
        nc.sync.dma_start(g1_sb[:], g1d[:])
        g2_sb = const_pool.tile([GROUPS, D], F32)
        nc.sync.dma_start(g2_sb[:], g2d[:])
        idx_sb = idx_pool.tile([2 * D, B * (J // 16)], I16)
        for b in range(B):
            nc.sync.dma_start(
                idx_sb[:, b * (J // 16) : (b + 1) * (J // 16)], idxg[b]
            )

        b_col = misc_sb[:, 0:1]
        gam_col = misc_sb[:, 1:2]
        bet_col = misc_sb[:, 2:3]

        # per-(b,tile,chunk) stats columns: S = sum x, Q = sum x^2 per channel
        NCOL = B * NT * NC
        statsS = stat_pool.tile([D, NCOL], F32)
        statsQ = stat_pool.tile([D, NCOL], F32)
        dump = dump_pool.tile([D, CHUNK], F32)

        # ---------------- pass A: stats (no gathers in this stream) ------
        for b in range(B):
            for t in range(NT):
                jslc = slice(t * TILE, (t + 1) * TILE)
                vt = vt_pool.tile([21, TILE], F16, tag="vt")
                nc.sync.dma_start(vt[:, :], rhs[b, :, jslc])
                for c in range(NC):
                    ps = psum_pool.tile([D, CHUNK], F32, tag="ps")
                    for q in range(CHUNK // 512):
                        cq = slice(c * CHUNK + q * 512, c * CHUNK + (q + 1) * 512)
                        nc.tensor.matmul(
                            ps[:, q * 512 : (q + 1) * 512],
                            lhsT=wd_sb[:, :],
                            rhs=vt[:, cq],
                            start=True,
                            stop=True,
                        )
                    col = (b * NT + t) * NC + c
                    nc.vector.tensor_reduce(
                        statsS[:, col : col + 1],
                        ps[:, :],
                        axis=mybir.AxisListType.X,
                        op=mybir.AluOpType.add,
                    )
                    nc.scalar.activation(
                        dump[:, :],
                        ps[:, :],
                        mybir.ActivationFunctionType.Square,
                        accum_out=statsQ[:, col : col + 1],
                    )

        # ---------------- stats finalize ----------------
        sqy = stat_pool.tile([D, 4], F32)  # cols: S_b0, S_b1, Q_b0, Q_b1 (local)
        for b in range(B):
            csl = slice(b * NT * NC, (b + 1) * NT * NC)
            nc.vector.tensor_reduce(
                sqy[:, b : b + 1], statsS[:, csl],
                axis=mybir.AxisListType.X, op=mybir.AluOpType.add,
            )
            nc.vector.tensor_reduce(
                sqy[:, 2 + b : 3 + b], statsQ[:, csl],
                axis=mybir.AxisListType.X, op=mybir.AluOpType.add,
            )
        arin = dram_pool.tile([D, 4], F32)
        arout = dram_pool.tile([D, 4], F32)
        nc.sync.dma_start(arin[:], sqy[:, :])

        # ------- feature-gather stream (Pool + Sync queues, independent) --
        # The AllReduce trigger is interleaved into the gather stream so the
        # Pool queue reaches it right around the time the stats finish; pass B
        # (on the Act HWDGE queue) then overlaps the remaining gathers.
        GC = min(GATHER_CHUNK, TILE)  # max num_idxs per dma_gather on this hw
        GPTS = GC // K
        NGC = J // GC
        CC_AT = 56  # gather calls issued before the AllReduce trigger
        gcalls = [(b, g) for b in range(B) for g in range(NGC)]

        def emit_gather(b, g):
            i0 = b * (J // 16) + g * (GC // 16)
            gth = gath_pool.tile([2 * D, GC], F16, tag="gth")
            nc.gpsimd.dma_gather(
                gth[:, :].rearrange("p (o n) -> p o n", o=1),
                fp[:, :],
                idx_sb[:, i0 : i0 + GC // 16],
                num_idxs=GC,
                num_idxs_reg=GC,
                elem_size=2 * D,
                transpose=True,
            )
            nc.sync.dma_start(
                out[b, D : 2 * D, g * GPTS : (g + 1) * GPTS, :],
                gth[b * D : (b + 1) * D, :],
            )

        for b, g in gcalls[:CC_AT]:
            emit_gather(b, g)
        nc.gpsimd.collective_compute(
            "AllReduce",
            mybir.AluOpType.add,
            replica_groups=[list(range(n_cores))],
            ins=[arin.opt()],
            outs=[arout.opt()],
        )
        for b, g in gcalls[CC_AT:]:
            emit_gather(b, g)

        sq_g = stat_pool.tile([D, 4], F32)  # global S_b0, S_b1, Q_b0, Q_b1
        nc.scalar.dma_start(sq_g[:], arout[:])

        # with bias folded:  Sy = S + M*b ; Qy = Q + b*(M*b + 2S)
        sqy2 = stat_pool.tile([D, 4], F32)  # Sy_b0, Sy_b1, Qy_b0, Qy_b1
        s2 = stat_pool.tile([D, 2], F32)
        tmp1 = stat_pool.tile([D, 2], F32)
        for b in range(B):
            S_b = sq_g[:, b : b + 1]
            Q_b = sq_g[:, 2 + b : 3 + b]
            nc.scalar.activation(
                sqy2[:, b : b + 1], b_col,
                mybir.ActivationFunctionType.Identity, bias=S_b, scale=MTOT,
            )
            nc.vector.tensor_add(s2[:, b : b + 1], S_b, S_b)
            nc.scalar.activation(
                tmp1[:, b : b + 1], b_col,
                mybir.ActivationFunctionType.Identity,
                bias=s2[:, b : b + 1], scale=MTOT,
            )
            nc.vector.tensor_mul(tmp1[:, b : b + 1], tmp1[:, b : b + 1], b_col)
            nc.vector.tensor_add(sqy2[:, 2 + b : 3 + b], Q_b, tmp1[:, b : b + 1])

        # group sums: gs[16, 4] = g1^T @ sqy2
        gps = psum_pool.tile([GROUPS, 4], F32, tag="ps")
        nc.tensor.matmul(gps[:, :], lhsT=g1_sb[:, :], rhs=sqy2[:, :], start=True, stop=True)
        mue = stat_pool.tile([GROUPS, 4], F32)  # cols 0-1: mu; 2-3: E2 then rs
        inv4m = 1.0 / (4.0 * MTOT)
        nc.scalar.activation(mue[:, :], gps[:, :], mybir.ActivationFunctionType.Copy, scale=inv4m)
        musq = stat_pool.tile([GROUPS, 2], F32)
        nc.scalar.activation(musq[:, :], mue[:, 0:2], mybir.ActivationFunctionType.Square)
        var = stat_pool.tile([GROUPS, 2], F32)
        nc.vector.tensor_sub(var[:, :], mue[:, 2:4], musq[:, :])
        nc.vector.tensor_scalar_add(var[:, :], var[:, :], EPS)
        nc.vector.reciprocal(var[:, :], var[:, :])
        nc.scalar.activation(mue[:, 2:4], var[:, :], mybir.ActivationFunctionType.Sqrt)

        # broadcast groups -> channels: mr64[64, 4] = g2^T @ mue
        mps = psum_pool.tile([D, 4], F32, tag="ps")
        nc.tensor.matmul(mps[:, :], lhsT=g2_sb[:, :], rhs=mue[:, :], start=True, stop=True)
        mr64 = stat_pool.tile([D, 4], F32)
        nc.scalar.activation(mr64[:, :], mps[:, :], mybir.ActivationFunctionType.Copy)

        # final per-channel scale s = gamma*rs, shift t = (b - mu)*s + beta
        sc = stat_pool.tile([D, 2], F32)
        tc_ = stat_pool.tile([D, 2], F32)
        for b in range(B):
            nc.vector.tensor_mul(sc[:, b : b + 1], mr64[:, 2 + b : 3 + b], gam_col)
            nc.vector.tensor_sub(tc_[:, b : b + 1], b_col, mr64[:, b : b + 1])
            nc.vector.tensor_mul(tc_[:, b : b + 1], tc_[:, b : b + 1], sc[:, b : b + 1])
            nc.vector.tensor_add(tc_[:, b : b + 1], tc_[:, b : b + 1], bet_col)

        # ---------------- pass B: x = relu(s*x_raw + t) ----------------
        # All pass-B DMAs ride the Act HWDGE queue: the Sync queue is busy
        # draining gather->out writes and would head-of-line block pass B.
        for b in range(B):
            for t in range(NT):
                jslc = slice(t * TILE, (t + 1) * TILE)
                vt = vt_pool.tile([21, TILE], F16, tag="vt")
                nc.scalar.dma_start(vt[:, :], rhs[b, :, jslc])
                xo = xo_pool.tile([D, TILE], F16, tag="xo")
                for c in range(NC):
                    ps = psum_pool.tile([D, CHUNK], F32, tag="ps")
                    for q in range(CHUNK // 512):
                        cq = slice(c * CHUNK + q * 512, c * CHUNK + (q + 1) * 512)
                        nc.tensor.matmul(
                            ps[:, q * 512 : (q + 1) * 512],
                            lhsT=wd_sb[:, :],
                            rhs=vt[:, cq],
                            start=True,
                            stop=True,
                        )
                    nc.scalar.activation(
                        xo[:, c * CHUNK : (c + 1) * CHUNK],
                        ps[:, :],
                        mybir.ActivationFunctionType.Relu,
                        bias=tc_[:, b : b + 1],
                        scale=sc[:, b : b + 1],
                    )
                nc.scalar.dma_start(
                    out[b, 0:D, t * PTS : (t + 1) * PTS, :], xo[:, :]
                )

    nc.compile()
    return nc


def host_prep(coords, features, idx, dist, conv_w, conv_b, gn_gamma, gn_beta,
              N, NS, K, n_cores):
    """Full inputs -> list of per-core input maps."""
    coords = np.asarray(coords, dtype=np.float32)
    features = np.asarray(features, dtype=np.float32)
    idx = np.asarray(idx)
    dist = np.asarray(dist, dtype=np.float32)
    conv_w = np.asarray(conv_w, dtype=np.float32)
    conv_b = np.asarray(conv_b, dtype=np.float32)
    gn_gamma = np.asarray(gn_gamma, dtype=np.float32)
    gn_beta = np.asarray(gn_beta, dtype=np.float32)

    J = NS * K
    # point-major f16 feature plane: row n = [b0 features, b1 features]
    fp = np.ascontiguousarray(
        features[:, :, :, 0].astype(np.float16).transpose(2, 0, 1).reshape(N, 2 * D)
    )

    # weights: A = Wc + Wd, Bm = Wg - Wd, w9; lhsT rows = [A; Bm; w9]
    # matching the rhs row order [center(3); nbr(3); dist(1)]
    A = conv_w[:, 0:3] + conv_w[:, 6:9]
    Bm = conv_w[:, 3:6] - conv_w[:, 6:9]
    w9 = conv_w[:, 9:10]
    wb = np.concatenate([A.T, Bm.T, w9.T], axis=0).astype(np.float32)  # [7, 64]
    wh = wb.astype(np.float16)
    wl = (wb - wh.astype(np.float32)).astype(np.float16)
    wd21 = np.concatenate([wh, wl, wh], axis=0)  # [21, 64] f16

    misc = np.stack(
        [conv_b, gn_gamma, gn_beta, np.zeros_like(conv_b)], axis=1
    ).astype(np.float32)  # [64, 4]
    dgrp = np.arange(D) // (D // GROUPS)
    g1 = (dgrp[:, None] == np.arange(GROUPS)[None, :]).astype(np.float32)
    g2 = np.ascontiguousarray(g1.T)

    in_maps = []
    for c in range(n_cores):
        nsl = slice(c * NS, (c + 1) * NS)
        idx_c = idx[:, nsl, :]  # [B, NS, K]
        # wrapped int16 idx layout: index j at [j%16, j//16], replicated 8x
        idxf = idx_c.reshape(B, J)
        idxw16 = idxf.reshape(B, J // 16, 16).transpose(0, 2, 1).astype(np.int16)
        idxg = np.ascontiguousarray(np.tile(idxw16, (1, (2 * D) // 16, 1)))

        # exact f16 hi/lo split of the 7-row matmul rhs [c(3); g(3); d(1)],
        # duplicated into the 21-row [v_hi; v_hi; v_lo] matmul layout
        rhs21 = np.empty((B, 21, J), np.float16)
        for b in range(B):
            ctr = np.repeat(coords[b, nsl, :], K, axis=0).T      # [3, J]
            g = coords[b, idx_c[b].reshape(-1), :].T             # [3, J]
            d = dist[b, nsl, :].reshape(1, J)                    # [1, J]
            v7 = np.concatenate([ctr, g, d], axis=0)             # [7, J] f32
            vh = v7.astype(np.float16)
            vl = (v7 - vh.astype(np.float32)).astype(np.float16)
            rhs21[b, 0:7] = vh
            rhs21[b, 7:14] = vh
            rhs21[b, 14:21] = vl

        in_maps.append(
            {
                "fp": fp,
                "idxg": idxg,
                "rhs21": rhs21,
                "wd21": wd21,
                "misc": misc,
                "g1": g1,
                "g2": g2,
            }
        )
    return in_maps


def assemble(results, N, NS, K, n_cores):
    """Per-core 'out' shards (f16) -> full (B, 2D, N, K) f32."""
    return np.concatenate(
        [results[c]["out"] for c in range(n_cores)], axis=2
    ).astype(np.float32)


# ---------------------------------------------------------------------------
# self-contained entry point: full inputs -> full output on 8 NeuronCores
# ---------------------------------------------------------------------------
_N, _NS, _K, _TILE, _NCORES = 32768, 4096, 16, 2048, 8
_PROGRAM = None


def _get_program():
    global _PROGRAM
    if _PROGRAM is None:
        _PROGRAM = build_program(_N, _NS, _K, _TILE, _NCORES)
    return _PROGRAM


def kernel(coords, features, idx, dist, conv_w, conv_b, gn_gamma, gn_beta):
    nc = _get_program()
    in_maps = host_prep(
        coords, features, idx, dist, conv_w, conv_b, gn_gamma, gn_beta,
        _N, _NS, _K, _NCORES,
    )
    from concourse.bass_utils import run_bass_kernel_spmd

    res = run_bass_kernel_spmd(nc, in_maps, list(range(_NCORES)))
    return assemble(res.results, _N, _NS, _K, _NCORES)
